# revision 9
# baseline (speedup 1.0000x reference)
"""Cross-attention (causal) Trainium2 kernel, 8-core SPMD, bf16 compute.

Sharding: core c -> batch c//2, decoder-row half c%2.  Half 0 owns 128-row
q-blocks {0,3,4,7}, half 1 owns {1,2,5,6} of T_dec=1024 (balances causal
work); zero collectives, host reassembles disjoint output rows.

Host prep: X and W pre-transposed to emb-major bf16; bv folded into the
output-projection bias (softmax weights sum to 1); masks/biases arranged
for contiguous DMA.  bf16 matmuls, fp32 PSUM (rel err ~2.4e-3, gate 2e-2).

Single fused pipeline: the Q/K/V projection units are interleaved into the
attention head-pair loop so the exp-bound slack absorbs them — pair p
consumes QT[p]/KT[p] produced one pair earlier; V ch0 units feed pair 0's
j-loop just-in-time, ch1 lands during pairs 1-2 (needed from pair 4).
Weights stream d-chunked so the first units are not DMA-gated.  Heads
2t/2t+1 run on PE row-groups 0-63/64-127 (concurrent S into the two banks
of one 2-bank st tile); exp and mask cover both heads in single strided
ops.  Softmax tail: copy av->SBUF (frees psum), l via DMA to partition 0,
gpsimd broadcast, reciprocal on 64 lanes, multiply.
"""

import numpy as np

P = 128
DE = 1024          # emb dim
Q = 512            # q rows per core
Q2 = 2 * Q
H = 16
HD = 64
ET = DE // P       # 8 e-tiles
# active q-cols per key-block j (shared max over both halves); the causal
# mask window is always cols [0:128) of the active suffix.
N_J = [512, 512, 384, 384, 256, 256, 128, 128]
QB = ([0, 3, 4, 7], [1, 2, 5, 6])                # q-block assignment per half

_NC_CACHE = {}


def _build_nc():
    import concourse.tile as tile
    from concourse import bacc, mybir

    F32 = mybir.dt.float32
    BF16 = mybir.dt.bfloat16
    AF = mybir.ActivationFunctionType

    nc = bacc.Bacc("TRN2", target_bir_lowering=False, debug=False)

    xdT = nc.dram_tensor("xdT", [DE, Q], BF16, kind="ExternalInput").ap()
    xeT = nc.dram_tensor("xeT", [DE, DE], BF16, kind="ExternalInput").ap()
    wqT = nc.dram_tensor("wqT", [DE, DE], BF16, kind="ExternalInput").ap()
    wkT = nc.dram_tensor("wkT", [DE, DE], BF16, kind="ExternalInput").ap()
    wvT = nc.dram_tensor("wvT", [DE, DE], BF16, kind="ExternalInput").ap()
    wpT = nc.dram_tensor("wpT", [DE, DE], BF16, kind="ExternalInput").ap()
    bqp = nc.dram_tensor("bqp", [P, ET], F32, kind="ExternalInput").ap()
    bkp = nc.dram_tensor("bkp", [P, ET], F32, kind="ExternalInput").ap()
    cb = nc.dram_tensor("cb", [DE], BF16, kind="ExternalInput").ap()
    masks2 = nc.dram_tensor("masks2", [P, 8, 2, P], BF16,
                            kind="ExternalInput").ap()
    out = nc.dram_tensor("out", [Q, DE], F32, kind="ExternalOutput").ap()

    with tile.TileContext(nc) as tc:
        with tc.tile_pool(name="persist", bufs=1) as pp, \
             tc.tile_pool(name="consts", bufs=1) as cp:
            XdT = [pp.tile([P, Q], BF16, name=f"XdT{i}") for i in range(ET)]
            XeT = [pp.tile([P, DE], BF16, name=f"XeT{i}") for i in range(ET)]
            WQ = [pp.tile([P, DE], BF16, name=f"WQ{i}") for i in range(ET)]
            WK = [pp.tile([P, DE], BF16, name=f"WK{i}") for i in range(ET)]
            WV = [pp.tile([P, DE], BF16, name=f"WV{i}") for i in range(ET)]
            WP = [pp.tile([P, DE], BF16, name=f"WP{i}") for i in range(ET)]
            QT = [pp.tile([P, Q], BF16, name=f"QT{i}") for i in range(ET)]
            KT = [pp.tile([P, DE], BF16, name=f"KT{i}") for i in range(ET)]
            VA = [pp.tile([P, H * (HD + 1)], BF16, name=f"VA{i}")
                  for i in range(ET)]
            YT = [pp.tile([P, Q], BF16, name=f"YT{i}") for i in range(ET)]

            # DMA streams, ordered so each pipeline unit's operands land
            # just-in-time: XdT, WQ[d0], XeT.ch0, WK[d0], WV.ch0, XeT.ch1,
            # then remaining W d-chunks, WV.ch1, consts, WP.
            def wchunk(dst, src, d):
                for e in range(ET):
                    nc.sync.dma_start(
                        out=dst[e][:, d * P:(d + 1) * P],
                        in_=src[e * P:(e + 1) * P, d * P:(d + 1) * P])

            for e in range(ET):
                nc.sync.dma_start(out=XdT[e], in_=xdT[e * P:(e + 1) * P, :])
            wchunk(WQ, wqT, 0)
            for e in range(ET):
                nc.sync.dma_start(out=XeT[e][:, :Q],
                                  in_=xeT[e * P:(e + 1) * P, :Q])
            wchunk(WK, wkT, 0)
            for e in range(ET):
                nc.sync.dma_start(out=WV[e][:, :Q],
                                  in_=wvT[e * P:(e + 1) * P, :Q])
            for e in range(ET):
                nc.sync.dma_start(out=XeT[e][:, Q:],
                                  in_=xeT[e * P:(e + 1) * P, Q:])
            for d in range(1, ET):
                wchunk(WQ, wqT, d)
                wchunk(WK, wkT, d)
            for e in range(ET):
                nc.sync.dma_start(out=WV[e][:, Q:],
                                  in_=wvT[e * P:(e + 1) * P, Q:])

            ones16 = cp.tile([P, H], BF16)
            nc.vector.memset(ones16, 1.0)
            ones1 = cp.tile([1, P], BF16)
            nc.vector.memset(ones1, 1.0)
            bq_sb = cp.tile([P, ET], F32)
            nc.gpsimd.dma_start(out=bq_sb, in_=bqp)
            bk_sb = cp.tile([P, ET], F32)
            nc.gpsimd.dma_start(out=bk_sb, in_=bkp)
            cb_row = cp.tile([1, DE], BF16)
            nc.gpsimd.dma_start(out=cb_row, in_=cb[None, :])
            masks_sb = cp.tile([P, 8, 2, P], BF16)
            nc.gpsimd.dma_start(out=masks_sb, in_=masks2)

            for e in range(ET):
                nc.sync.dma_start(out=WP[e], in_=wpT[e * P:(e + 1) * P, :])

            with tc.tile_pool(name="pt", bufs=3) as ptp, \
                 tc.tile_pool(name="ysb", bufs=2) as ysbp, \
                 tc.tile_pool(name="sm", bufs=3) as smp, \
                 tc.tile_pool(name="osb", bufs=2) as osbp, \
                 tc.tile_pool(name="ps", bufs=2, space="PSUM") as ps, \
                 tc.tile_pool(name="st", bufs=2, space="PSUM") as stp, \
                 tc.tile_pool(name="av", bufs=1, space="PSUM") as avp:

                def q_unit(d):
                    psq = ps.tile([P, Q], F32, tag="ps")
                    for e in range(ET):
                        nc.tensor.matmul(
                            psq[:], WQ[e][:, d * P:(d + 1) * P], XdT[e][:],
                            start=(e == 0), stop=(e == ET - 1))
                    nc.scalar.activation(QT[d][:], psq[:], AF.Identity,
                                         bias=bq_sb[:, d:d + 1])

                def k_unit(d, ch):
                    psk = ps.tile([P, Q], F32, tag="ps")
                    for e in range(ET):
                        nc.tensor.matmul(
                            psk[:], WK[e][:, d * P:(d + 1) * P],
                            XeT[e][:, ch * Q:(ch + 1) * Q],
                            start=(e == 0), stop=(e == ET - 1))
                    dst = KT[d][:, ch * Q:(ch + 1) * Q]
                    if ch == 0:
                        nc.scalar.activation(dst, psk[:], AF.Identity,
                                             bias=bk_sb[:, d:d + 1])
                    else:
                        nc.vector.tensor_scalar_add(dst, psk[:],
                                                    bk_sb[:, d:d + 1])

                def v_unit(kt, ch):
                    psv = ps.tile([P, Q], F32, tag="ps")
                    for e in range(ET):
                        nc.tensor.matmul(
                            psv[:], XeT[e][:, kt * P:(kt + 1) * P],
                            WV[e][:, ch * Q:(ch + 1) * Q],
                            start=(e == 0), stop=(e == ET - 1))
                    hbase = 8 * ch
                    dst = VA[kt][:, hbase * (HD + 1):(hbase + 8) * (HD + 1)]
                    dst = dst.rearrange("p (h x) -> p h x", h=8)[:, :, :HD]
                    nc.vector.tensor_copy(
                        dst, psv.rearrange("p (h x) -> p h x", h=8))
                    if ch == 0:
                        onesdst = VA[kt].rearrange(
                            "p (h x) -> p h x", x=HD + 1)[:, :, HD:HD + 1]
                        nc.vector.tensor_copy(
                            onesdst, ones16.rearrange("p (h x) -> p h x", x=1))

                def emit_s(ht, j):
                    nj = N_J[j]
                    cs = P * (j // 2)
                    st = stp.tile([P, Q2], F32, tag="st")
                    for i, off in enumerate((0, HD)):
                        nc.tensor.matmul(
                            st[:, i * Q:i * Q + nj],
                            KT[ht][off:off + HD, j * P:(j + 1) * P],
                            QT[ht][off:off + HD, cs:cs + nj],
                            start=True, stop=True)
                    pt = ptp.tile([P, Q2], BF16, tag="pt")
                    st_v = st.rearrange("p (b c) -> p b c", b=2)[:, :, :nj]
                    pt_v = pt.rearrange("p (b c) -> p b c", b=2)[:, :, :nj]
                    nc.scalar.activation(pt_v, st_v, AF.Exp, scale=0.125)
                    pt_m = pt.rearrange("p (b c) -> p b c", b=2)[:, :, :P]
                    nc.vector.tensor_mul(pt_m, pt_m, masks_sb[:, j])
                    return pt

                def emit_av(ht, j, pt, av):
                    nj = N_J[j]
                    cs = P * (j // 2)
                    for i, h in enumerate((2 * ht, 2 * ht + 1)):
                        nc.tensor.matmul(
                            av[:, i * Q + cs:i * Q + cs + nj],
                            VA[j][:, h * (HD + 1):(h + 1) * (HD + 1)],
                            pt[:, i * Q:i * Q + nj],
                            start=(j == 0), stop=(j == 7))

                def units_for(p, j):
                    if p == 0 and j == 0:
                        k_unit(0, 1)
                    if p == 0 and j < 7:
                        v_unit(j + 1, 0)
                    if p < 7:
                        if j == 0:
                            q_unit(p + 1)
                        elif j == 1:
                            k_unit(p + 1, 0)
                        elif j == 5:
                            k_unit(p + 1, 1)
                    if p in (1, 2) and j in (0, 2, 4, 6):
                        v_unit(4 * (p - 1) + j // 2, 1)

                # pre-loop units feeding pair 0
                q_unit(0)
                k_unit(0, 0)
                v_unit(0, 0)

                for ht in range(ET):
                    av = avp.tile([HD + 1, Q2], F32, name=f"av{ht}", tag="av")
                    pend = emit_s(ht, 0)
                    for j in range(8):
                        units_for(ht, j)
                        nxt = emit_s(ht, j + 1) if j < 7 else None
                        emit_av(ht, j, pend, av)
                        pend = nxt
                    ysb = ysbp.tile([HD + 1, Q2], F32, name=f"ysb{ht}",
                                    tag="ysb")
                    if ht % 2 == 0:
                        nc.scalar.copy(ysb[:], av[:])
                    else:
                        nc.vector.tensor_copy(ysb[:], av[:])
                    lrow = smp.tile([1, Q2], F32, tag="lrow")
                    nc.gpsimd.dma_start(out=lrow, in_=ysb[HD:HD + 1, :])
                    lb = smp.tile([HD, Q2], F32, tag="lb")
                    nc.gpsimd.partition_broadcast(lb[:], lrow[:])
                    rb = smp.tile([HD, Q2], F32, tag="rb")
                    nc.vector.reciprocal_approx_fast(out=rb[:], in_=lb[:])
                    nc.vector.tensor_mul(YT[ht][0:HD, :], ysb[:HD, :Q],
                                         rb[:, :Q])
                    nc.vector.tensor_mul(YT[ht][HD:P, :], ysb[:HD, Q:],
                                         rb[:, Q:])

                # ---- output projection (reuses the ps pool) ------------
                for m in range(4):
                    osb = osbp.tile([P, DE], F32, tag="osb")
                    for ch in range(2):
                        pso = ps.tile([P, Q], F32, tag="ps")
                        nc.tensor.matmul(
                            pso[:], ones1[:], cb_row[:, ch * Q:(ch + 1) * Q],
                            start=True, stop=False)
                        for a in range(ET):
                            nc.tensor.matmul(
                                pso[:], YT[a][:, m * P:(m + 1) * P],
                                WP[a][:, ch * Q:(ch + 1) * Q],
                                start=False, stop=(a == ET - 1))
                        if ch == 0:
                            nc.scalar.copy(osb[:, :Q], pso[:])
                        else:
                            nc.vector.tensor_copy(osb[:, Q:], pso[:])
                    nc.sync.dma_start(out=out[m * P:(m + 1) * P, :],
                                      in_=osb[:])

    nc.compile()
    return nc


def get_nc():
    if "nc" not in _NC_CACHE:
        _NC_CACHE["nc"] = _build_nc()
    return _NC_CACHE["nc"]


def make_masks(qblocks):
    m = np.zeros((8, P, P), dtype=np.float32)
    for j in range(8):
        p = j // 2
        gq = P * qblocks[p] + np.arange(P)[None, :]
        gk = P * j + np.arange(P)[:, None]
        m[j] = (gk <= gq).astype(np.float32)
    return m


def shard_inputs(x_encoder, x_decoder, Wq, bq, Wk, bk, Wv, bv, Wp, bp):
    from ml_dtypes import bfloat16

    def bT(a):  # transpose + bf16, contiguous
        return np.ascontiguousarray(np.asarray(a, np.float32).T).astype(bfloat16)

    wqT, wkT, wvT, wpT = bT(Wq), bT(Wk), bT(Wv), bT(Wp)
    bqp = np.ascontiguousarray(np.asarray(bq, np.float32).reshape(ET, P).T)
    bkp = np.ascontiguousarray(np.asarray(bk, np.float32).reshape(ET, P).T)
    # bv rides through softmax (weights sum to 1): fold into out-proj bias
    cb = (np.asarray(bv, np.float32) @ np.asarray(Wp, np.float32).T
          + np.asarray(bp, np.float32)).astype(bfloat16)
    xeT = [bT(x_encoder[b]) for b in range(4)]

    msk = []
    for h in range(2):
        m = make_masks(QB[h]).astype(bfloat16)      # [8, P, P]
        m = np.transpose(m, (1, 0, 2))              # [P(r), 8(j), P(c)]
        m = np.stack([m, m], axis=2)                # [P, 8, 2, P]
        msk.append(np.ascontiguousarray(m))

    in_maps = []
    for core in range(8):
        b, half = core // 2, core % 2
        xd = np.concatenate(
            [np.asarray(x_decoder[b][P * t:P * (t + 1)], np.float32)
             for t in QB[half]], 0)
        in_maps.append({
            "xdT": bT(xd),  # [DE, Q]
            "xeT": xeT[b],
            "wqT": wqT, "wkT": wkT, "wvT": wvT, "wpT": wpT,
            "bqp": bqp, "bkp": bkp, "cb": cb,
            "masks2": msk[half],
        })
    return in_maps


def assemble(results, B=4, T=1024):
    out = np.zeros((B, T, DE), dtype=np.float32)
    for core in range(8):
        b, half = core // 2, core % 2
        for p, t in enumerate(QB[half]):
            out[b, P * t:P * (t + 1)] = results[core]["out"][P * p:P * (p + 1)]
    return out


def kernel(**inputs):
    from concourse.bass_utils import run_bass_kernel_spmd
    nc = get_nc()
    in_maps = shard_inputs(**{k: np.asarray(v) for k, v in inputs.items()})
    res = run_bass_kernel_spmd(nc, in_maps, core_ids=list(range(8)))
    return assemble(res.results)


if __name__ == "__main__":
    nc = get_nc()
    print("built + compiled ok")


# revision 10
# speedup vs baseline: 1.2019x; 1.2019x over previous
"""Cross-attention (causal) Trainium2 kernel, 8-core SPMD, bf16 compute.

Sharding: core c -> batch c//2, decoder-row half c%2.
Half 0 owns 128-row q-blocks {0,3,4,7}, half 1 owns {1,2,5,6} of T_dec=1024
(balances causal work at 18 key-block units each); zero collectives, host
reassembles disjoint output rows.

Host-side layout prep (shard_inputs): X and W are pre-transposed to
emb-major bf16; the V bias is folded into the output-projection bias
(softmax weights sum to 1, so  out = yhat@WpT + (bv@WpT + bp));  masks and
biases are pre-arranged for contiguous DMA.  All matmuls are bf16 with
fp32 PSUM accumulation (end-to-end rel err ~2.4e-3, gate 2e-2).

Per-core kernel:
  QT/KT channel-major via W^T-panel matmuls; V token-major, augmented with
  a per-head ones column so softmax denominators come free as row 64 of
  the AV psum.  Attention runs per head-PAIR: heads 2t/2t+1 occupy PE
  row-groups 0-63/64-127 (concurrent S matmuls into the two banks of one
  2-bank st tile), and exp + causal-mask ops cover both heads in single
  strided instructions.  N_J trims each key block to the causally active
  q-column suffix; the tri/zero/ones mask window is always the first 128
  columns (host-supplied per-core masks).  The softmax tail (1/l) runs
  bcast -> reciprocal on 64 lanes -> multiply, spread over gpsimd+vector
  and decoupled from the PSUM banks by an early copy to SBUF.
"""

import numpy as np

P = 128
DE = 1024          # emb dim
Q = 512            # q rows per core
Q2 = 2 * Q
H = 16
HD = 64
ET = DE // P       # 8 e-tiles
# active q-cols per key-block j (shared max over both halves); the causal
# suffix starts at column 128*(j//2), so the mask window is always cols
# [0:128) of the active slice.
N_J = [512, 512, 384, 384, 256, 256, 128, 128]
QB = ([0, 3, 4, 7], [1, 2, 5, 6])                # q-block assignment per half

_NC_CACHE = {}


def _build_nc():
    import concourse.tile as tile
    from concourse import bacc, mybir

    F32 = mybir.dt.float32
    BF16 = mybir.dt.bfloat16
    AF = mybir.ActivationFunctionType

    nc = bacc.Bacc("TRN2", target_bir_lowering=False, debug=False)

    xdT = nc.dram_tensor("xdT", [DE, Q], BF16, kind="ExternalInput").ap()
    xeT = nc.dram_tensor("xeT", [DE, DE], BF16, kind="ExternalInput").ap()
    wqT = nc.dram_tensor("wqT", [DE, DE], BF16, kind="ExternalInput").ap()
    wkT = nc.dram_tensor("wkT", [DE, DE], BF16, kind="ExternalInput").ap()
    wvT = nc.dram_tensor("wvT", [DE, DE], BF16, kind="ExternalInput").ap()
    wpT = nc.dram_tensor("wpT", [DE, DE], BF16, kind="ExternalInput").ap()
    bqp = nc.dram_tensor("bqp", [P, ET], F32, kind="ExternalInput").ap()
    bkp = nc.dram_tensor("bkp", [P, ET], F32, kind="ExternalInput").ap()
    cb = nc.dram_tensor("cb", [DE], BF16, kind="ExternalInput").ap()
    masks2 = nc.dram_tensor("masks2", [P, 8, 2, P], BF16,
                            kind="ExternalInput").ap()
    out = nc.dram_tensor("out", [Q, DE], F32, kind="ExternalOutput").ap()

    with tile.TileContext(nc) as tc:
        with tc.tile_pool(name="persist", bufs=1) as pp, \
             tc.tile_pool(name="consts", bufs=1) as cp:
            # persistent activations + weight panels (bf16, emb-major)
            XdT = [pp.tile([P, Q], BF16, name=f"XdT{i}") for i in range(ET)]
            XeT = [pp.tile([P, DE], BF16, name=f"XeT{i}") for i in range(ET)]
            WQ = [pp.tile([P, DE], BF16, name=f"WQ{i}") for i in range(ET)]
            WK = [pp.tile([P, DE], BF16, name=f"WK{i}") for i in range(ET)]
            WV = [pp.tile([P, DE], BF16, name=f"WV{i}") for i in range(ET)]
            WP = [pp.tile([P, DE], BF16, name=f"WP{i}") for i in range(ET)]
            QT = [pp.tile([P, Q], BF16, name=f"QT{i}") for i in range(ET)]
            KT = [pp.tile([P, DE], BF16, name=f"KT{i}") for i in range(ET)]
            VA = [pp.tile([P, H * (HD + 1)], BF16, name=f"VA{i}")
                  for i in range(ET)]
            YT = [pp.tile([P, Q], BF16, name=f"YT{i}") for i in range(ET)]

            # DMA prefetch in consumption order: (XdT, WQ) -> (XeT, WK)
            # -> WV -> consts -> WP.
            for e in range(ET):
                nc.sync.dma_start(out=XdT[e], in_=xdT[e * P:(e + 1) * P, :])
                nc.sync.dma_start(out=WQ[e], in_=wqT[e * P:(e + 1) * P, :])
            for e in range(ET):
                nc.sync.dma_start(out=XeT[e], in_=xeT[e * P:(e + 1) * P, :])
                nc.sync.dma_start(out=WK[e], in_=wkT[e * P:(e + 1) * P, :])
            for e in range(ET):
                nc.sync.dma_start(out=WV[e], in_=wvT[e * P:(e + 1) * P, :])

            ones1 = cp.tile([1, P], BF16)
            nc.vector.memset(ones1, 1.0)
            ones16 = cp.tile([P, H], BF16)
            nc.vector.memset(ones16, 1.0)
            bq_sb = cp.tile([P, ET], F32)
            nc.gpsimd.dma_start(out=bq_sb, in_=bqp)
            bk_sb = cp.tile([P, ET], F32)
            nc.gpsimd.dma_start(out=bk_sb, in_=bkp)
            cb_row = cp.tile([1, DE], BF16)
            nc.gpsimd.dma_start(out=cb_row, in_=cb[None, :])
            masks_sb = cp.tile([P, 8, 2, P], BF16)
            nc.gpsimd.dma_start(out=masks_sb, in_=masks2)

            for e in range(ET):
                nc.sync.dma_start(out=WP[e], in_=wpT[e * P:(e + 1) * P, :])

            with tc.tile_pool(name="pt", bufs=3) as ptp, \
                 tc.tile_pool(name="ysb", bufs=2) as ysbp, \
                 tc.tile_pool(name="sm", bufs=3) as smp, \
                 tc.tile_pool(name="osb", bufs=2) as osbp:

                # ---- Q / K / V projections (3 PSUM banks) ---------------
                with tc.tile_pool(name="ps", bufs=3, space="PSUM") as ps:
                    for d in range(ET):
                        psq = ps.tile([P, Q], F32, tag="ps")
                        for e in range(ET):
                            nc.tensor.matmul(
                                psq[:], WQ[e][:, d * P:(d + 1) * P], XdT[e][:],
                                start=(e == 0), stop=(e == ET - 1))
                        nc.scalar.activation(QT[d][:], psq[:], AF.Identity,
                                             bias=bq_sb[:, d:d + 1])
                    for d in range(ET):
                        for ch in range(2):
                            psk = ps.tile([P, Q], F32, tag="ps")
                            for e in range(ET):
                                nc.tensor.matmul(
                                    psk[:], WK[e][:, d * P:(d + 1) * P],
                                    XeT[e][:, ch * Q:(ch + 1) * Q],
                                    start=(e == 0), stop=(e == ET - 1))
                            nc.scalar.activation(
                                KT[d][:, ch * Q:(ch + 1) * Q], psk[:],
                                AF.Identity, bias=bk_sb[:, d:d + 1])
                    # V token-major; bv is folded into the out-proj bias.
                    for kt in range(ET):
                        for ch in range(2):
                            psv = ps.tile([P, Q], F32, tag="ps")
                            for e in range(ET):
                                nc.tensor.matmul(
                                    psv[:], XeT[e][:, kt * P:(kt + 1) * P],
                                    WV[e][:, ch * Q:(ch + 1) * Q],
                                    start=(e == 0), stop=(e == ET - 1))
                            hbase = 8 * ch
                            dst = VA[kt][:, hbase * (HD + 1):
                                         (hbase + 8) * (HD + 1)]
                            dst = dst.rearrange(
                                "p (h x) -> p h x", h=8)[:, :, :HD]
                            src = psv.rearrange("p (h x) -> p h x", h=8)
                            nc.vector.tensor_copy(dst, src)
                        onesdst = VA[kt].rearrange(
                            "p (h x) -> p h x", x=HD + 1)[:, :, HD:HD + 1]
                        nc.vector.tensor_copy(
                            onesdst, ones16.rearrange("p (h x) -> p h x", x=1))

                # ---- attention, one head-pair per 2-bank st/av tile -----
                with tc.tile_pool(name="st", bufs=2, space="PSUM") as stp, \
                     tc.tile_pool(name="av", bufs=2, space="PSUM") as avp:

                    def emit_s(ht, j):
                        nj = N_J[j]
                        cs = P * (j // 2)
                        st = stp.tile([P, Q2], F32, tag="st")
                        for i, off in enumerate((0, HD)):
                            nc.tensor.matmul(
                                st[:, i * Q:i * Q + nj],
                                KT[ht][off:off + HD, j * P:(j + 1) * P],
                                QT[ht][off:off + HD, cs:cs + nj],
                                start=True, stop=True)
                        pt = ptp.tile([P, Q2], BF16, tag="pt")
                        st_v = st.rearrange("p (b c) -> p b c", b=2)[:, :, :nj]
                        pt_v = pt.rearrange("p (b c) -> p b c", b=2)[:, :, :nj]
                        nc.scalar.activation(pt_v, st_v, AF.Exp, scale=0.125)
                        pt_m = pt.rearrange("p (b c) -> p b c", b=2)[:, :, :P]
                        nc.vector.tensor_mul(pt_m, pt_m, masks_sb[:, j])
                        return pt

                    def emit_av(ht, j, pt, av):
                        nj = N_J[j]
                        cs = P * (j // 2)
                        for i, h in enumerate((2 * ht, 2 * ht + 1)):
                            nc.tensor.matmul(
                                av[:, i * Q + cs:i * Q + cs + nj],
                                VA[j][:, h * (HD + 1):(h + 1) * (HD + 1)],
                                pt[:, i * Q:i * Q + nj],
                                start=(j == 0), stop=(j == 7))

                    for ht in range(ET):
                        av = avp.tile([HD + 1, Q2], F32, name=f"av{ht}",
                                      tag="av")
                        pend = emit_s(ht, 0)
                        for j in range(8):
                            nxt = emit_s(ht, j + 1) if j < 7 else None
                            emit_av(ht, j, pend, av)
                            pend = nxt
                        # softmax tail: copy to SBUF (frees the psum banks),
                        # then bcast(l) -> 1/l on 64 lanes -> y*(1/l).
                        ysb = ysbp.tile([HD + 1, Q2], F32, name=f"ysb{ht}",
                                        tag="ysb")
                        nc.vector.tensor_copy(ysb[:], av[:])
                        # stage l on partition 0 (partition_broadcast reads
                        # the tile's partition 0 on hardware), via DMA ring
                        lrow = smp.tile([1, Q2], F32, tag="lrow")
                        nc.gpsimd.dma_start(out=lrow, in_=ysb[HD:HD + 1, :])
                        lb = smp.tile([HD, Q2], F32, tag="lb")
                        nc.gpsimd.partition_broadcast(lb[:], lrow[:])
                        rb = smp.tile([HD, Q2], F32, tag="rb")
                        nc.vector.reciprocal_approx_fast(out=rb[:], in_=lb[:])
                        nc.vector.tensor_mul(YT[ht][0:HD, :], ysb[:HD, :Q],
                                             rb[:, :Q])
                        nc.vector.tensor_mul(YT[ht][HD:P, :], ysb[:HD, Q:],
                                             rb[:, Q:])

                # ---- output projection ---------------------------------
                with tc.tile_pool(name="po", bufs=3, space="PSUM") as po:
                    for m in range(4):
                        osb = osbp.tile([P, DE], F32, tag="osb")
                        for ch in range(2):
                            pso = po.tile([P, Q], F32, tag="po")
                            nc.tensor.matmul(
                                pso[:], ones1[:],
                                cb_row[:, ch * Q:(ch + 1) * Q],
                                start=True, stop=False)
                            for a in range(ET):
                                nc.tensor.matmul(
                                    pso[:], YT[a][:, m * P:(m + 1) * P],
                                    WP[a][:, ch * Q:(ch + 1) * Q],
                                    start=False, stop=(a == ET - 1))
                            if ch == 0:
                                nc.scalar.copy(osb[:, :Q], pso[:])
                            else:
                                nc.vector.tensor_copy(osb[:, Q:], pso[:])
                        nc.sync.dma_start(out=out[m * P:(m + 1) * P, :],
                                          in_=osb[:])

    nc.compile()
    return nc


def get_nc():
    if "nc" not in _NC_CACHE:
        _NC_CACHE["nc"] = _build_nc()
    return _NC_CACHE["nc"]


def make_masks(qblocks):
    m = np.zeros((8, P, P), dtype=np.float32)
    for j in range(8):
        p = j // 2
        gq = P * qblocks[p] + np.arange(P)[None, :]
        gk = P * j + np.arange(P)[:, None]
        m[j] = (gk <= gq).astype(np.float32)
    return m


def shard_inputs(x_encoder, x_decoder, Wq, bq, Wk, bk, Wv, bv, Wp, bp):
    from ml_dtypes import bfloat16

    def bT(a):  # transpose + bf16, contiguous
        return np.ascontiguousarray(np.asarray(a, np.float32).T).astype(bfloat16)

    wqT, wkT, wvT, wpT = bT(Wq), bT(Wk), bT(Wv), bT(Wp)
    bqp = np.ascontiguousarray(
        np.asarray(bq, np.float32).reshape(ET, P).T)
    bkp = np.ascontiguousarray(
        np.asarray(bk, np.float32).reshape(ET, P).T)
    # bv rides through softmax (weights sum to 1): fold into out-proj bias
    cb = (np.asarray(bv, np.float32) @ np.asarray(Wp, np.float32).T
          + np.asarray(bp, np.float32)).astype(bfloat16)
    xeT = [bT(x_encoder[b]) for b in range(4)]

    msk = []
    for h in range(2):
        m = make_masks(QB[h]).astype(bfloat16)      # [8, P, P]
        m = np.transpose(m, (1, 0, 2))              # [P(r), 8(j), P(c)]
        m = np.stack([m, m], axis=2)                # [P, 8, 2, P]
        msk.append(np.ascontiguousarray(m))

    in_maps = []
    for core in range(8):
        b, half = core // 2, core % 2
        xd = np.concatenate(
            [np.asarray(x_decoder[b][P * t:P * (t + 1)], np.float32)
             for t in QB[half]], 0)
        in_maps.append({
            "xdT": bT(xd),  # [DE, Q]
            "xeT": xeT[b],
            "wqT": wqT, "wkT": wkT, "wvT": wvT, "wpT": wpT,
            "bqp": bqp, "bkp": bkp, "cb": cb,
            "masks2": msk[half],
        })
    return in_maps


def assemble(results, B=4, T=1024):
    out = np.zeros((B, T, DE), dtype=np.float32)
    for core in range(8):
        b, half = core // 2, core % 2
        for p, t in enumerate(QB[half]):
            out[b, P * t:P * (t + 1)] = results[core]["out"][P * p:P * (p + 1)]
    return out


def kernel(**inputs):
    from concourse.bass_utils import run_bass_kernel_spmd
    nc = get_nc()
    in_maps = shard_inputs(**{k: np.asarray(v) for k, v in inputs.items()})
    res = run_bass_kernel_spmd(nc, in_maps, core_ids=list(range(8)))
    return assemble(res.results)


if __name__ == "__main__":
    nc = get_nc()
    print("built + compiled ok")
